# revision 12
# baseline (speedup 1.0000x reference)
"""AdaAttN-style attention kernel for Trainium2, SPMD over 8 NeuronCores.

Math (C=256, N=8192, HW=96*96=9216):
    qn  = instnorm(q.T)                 # (C, N), stats over N
    qe  = qw @ qn + qb                  # (C, N)
    kn  = instnorm(st),  st = k[0]      # (C, HW), stats over HW
    ke  = kw @ kn + kb                  # (C, HW)
    se  = (sw @ st + sb).T              # (HW, C)
    A   = softmax(qe.T @ ke / sqrt(C))  # (N, HW)
    mean = A @ se; var = relu(A @ se^2 - mean^2)
    out = qn.T * sqrt(var) + mean       # (N, C)

Sharding: rows (N) split across the 8 cores; style side (k, weights) is
replicated and recomputed per core.  The q instance-norm statistics are the
only global coupling: shard-local bn_stats + a tiny gpsimd AllReduce of
[m_i, v_i + m_i^2].

Key algebra:
 1. Any per-row (per-n) additive constant in the logits cancels in softmax
    over hw.  Both the ke bias kb AND the k-side mean fold kw@(rs_k*m_k)
    enter the logits only as per-n constants, so they are dropped entirely.
 2. ke itself is never materialized: logits = qe.T@(kw' @ st)
    = (kw'.T @ qe).T @ st, so we fold the two 1x1 convs into one tiny
    M_T[c',c] = sum_co qw[co,c'] kw[co,c] (a 256x256 fp8 matmul of the raw
    weights, no stats dependency), then qk = rs_k * (M_T.T @ qsh + bias2),
    bias2 = kw.T@qb - M_T.T@(rs_q*m_q).  The big logits matmul then uses the
    already-resident st8 as the stationary operand.  This deletes the whole
    (C,HW) ke production + PSUM evacuation.

fp8 (TRN e4m3, max 240) everywhere on the attention path with DoubleRow
matmuls.  exp(SCALE*logit - 5.5) written straight to fp8 by ACT; the e^-5.5
cancels between softmax numerator and denominator.

Softmax denominator: per values-pair, 4 extra F=1 matmuls reusing the
already-loaded E_pair stationary, accumulated into 4 columns of ONE psum
bank.  Only the very first such matmul uses start=True (clears the bank's
pending-zero bits); every other stream's first write then sees its own
range's pending bit and overwrites, later ones accumulate.

Engine schedule: ACT evacuates W2 (se copies) during the 0-30us window
where DVE is stuck computing bn_stats; Pool squares se; everything lives in
ONE flat pool scope so there is no setup->main barrier.  One manual
LoadActFuncSet(natural_log_exp_and_others) up front keeps Exp/Ln/Copy in a
single table set -> no mid-kernel reloads.

PSUM banks: mv accumulators 4 + denominator 1 + shared 3-rotate
(M_T/se/bias/qk/qt/logits) = 8.
"""

import sys

if "/opt/trn_rl_repo" not in sys.path:
    sys.path.insert(0, "/opt/trn_rl_repo")

import numpy as np
import ml_dtypes

_f8_np = ml_dtypes.float8_e4m3
_bf16_np = ml_dtypes.bfloat16

C = 256
N = 8192
HW = 96 * 96  # 9216
NCORES = 8
NSH = N // NCORES  # 1024 rows per core
EPS = 1e-5
SCALE = C**-0.5
ESHIFT = 5.5  # exp(scale*logit - ESHIFT); cancels in softmax, keeps E < 240

CT = C // 128  # 2 channel tiles
HT = HW // 128  # 72 hw tiles
HP = HT // 2  # 36 hw tile pairs
NG = NSH // 512  # 2 n-groups per core
NJ = 4  # 4 row subtiles (128) per group
SCH = 1024  # st dma chunk (free dim)

ACT_SET_LN_EXP = 6  # natural_log_exp_and_others in act_info.json


def build(sim_local=False):
    import contextlib

    import concourse.bacc as bacc
    import concourse.bass as bass
    import concourse.tile as tile
    from concourse import mybir
    from concourse.masks import make_identity

    fp32 = mybir.dt.float32
    f8 = mybir.dt.float8e4
    AF = mybir.ActivationFunctionType
    ALU = mybir.AluOpType
    DR = mybir.MatmulPerfMode.DoubleRow

    nc = bacc.Bacc()

    qT_sh = nc.dram_tensor("qT_sh", [C, NSH], fp32, kind="ExternalInput")
    st_d = nc.dram_tensor("st", [C, HW], f8, kind="ExternalInput")
    qw8_d = nc.dram_tensor("qw8", [C, C], f8, kind="ExternalInput")
    kw8_d = nc.dram_tensor("kw8", [C, C], f8, kind="ExternalInput")
    swT_d = nc.dram_tensor("swT", [C, C], f8, kind="ExternalInput")
    qb_d = nc.dram_tensor("qb", [C], fp32, kind="ExternalInput")
    sb_d = nc.dram_tensor("sb", [C], fp32, kind="ExternalInput")
    out_d = nc.dram_tensor("out", [NSH, C], fp32, kind="ExternalOutput")
    cc_in = nc.dram_tensor("cc_in", [C, 2], fp32)
    cc_out = nc.dram_tensor("cc_out", [C, 2], fp32, addr_space="Shared")

    def bcast128(ap1d):
        return bass.AP(tensor=ap1d.tensor, offset=ap1d.offset, ap=[[0, 128], ap1d.ap[0]])

    with tile.TileContext(nc) as tc, contextlib.ExitStack() as ctx:
        # pin the one ACT table set (exp+ln+copy+square) for the whole kernel
        nc.scalar.add_instruction(
            mybir.InstLoadActFuncSet(
                name=nc.get_next_instruction_name(),
                act_func_set_id=ACT_SET_LN_EXP,
                ins=[],
                outs=[],
            )
        )

        consts = ctx.enter_context(tc.tile_pool(name="consts", bufs=1))
        kside = ctx.enter_context(tc.tile_pool(name="kside", bufs=1))
        setup = ctx.enter_context(tc.tile_pool(name="setup", bufs=2))
        # PSUM: one flat scope, no pool-release walls.
        #   ps_rot: shared 3-bank rotation (M_T, se, bias chains, qk, qt, logits)
        #   denps : softmax denominator columns
        #   mvps  : 4 mean/var accumulator banks
        ps_rot = ctx.enter_context(tc.tile_pool(name="ps_rot", bufs=3, space="PSUM"))
        denps = ctx.enter_context(tc.tile_pool(name="denps", bufs=1, space="PSUM"))
        mvps = ctx.enter_context(tc.tile_pool(name="mvps", bufs=1, space="PSUM"))
        epool = ctx.enter_context(tc.tile_pool(name="epool", bufs=4))
        ep = ctx.enter_context(tc.tile_pool(name="ep", bufs=2))
        outp = ctx.enter_context(tc.tile_pool(name="outp", bufs=2))

        ident = consts.tile([128, 128], fp32)
        ones1 = consts.tile([128, CT, 1], f8)  # denominator rhs (DR moving)
        eps_t = consts.tile([128, 1], fp32)
        nsh_t = consts.tile([128, 1], fp32)  # -ESHIFT for the exp bias
        sb_bc = consts.tile([128, C], fp32)
        qb_sb = consts.tile([128, CT], fp32)
        qb8 = consts.tile([128, CT, 1], f8)

        # ---- resident tensors ----
        st8 = kside.tile([128, CT, HW], f8)  # 18KB/part
        W2 = kside.tile([128, HP, 2, 512], f8)  # [se | se^2] per hw tile, 36KB/part
        qk8 = kside.tile([128, CT, NSH], f8)  # folded-weights q embedding
        qn_nat = kside.tile([128, NSH // 128, C], fp32)  # (n%128, n//128, c)
        MT8 = kside.tile([128, CT, C], f8)  # M_T[c'(part), c] = sum_co qw[co,c']kw[co,c]
        qw8 = kside.tile([128, CT, C], f8)
        kw8 = kside.tile([128, CT, C], f8)
        swT8 = kside.tile([128, CT, C], f8)

        stat_q = kside.tile([128, CT, NSH // 512, 6], fp32)
        stat_k = kside.tile([128, CT, HW // 512, 6], fp32)
        mv_q = kside.tile([128, CT, 2], fp32)
        mv_k = kside.tile([128, CT, 2], fp32)
        rs_q = kside.tile([128, CT], fp32)
        rs_k = kside.tile([128, CT], fp32)
        mqs8 = kside.tile([128, CT, 1], f8)  # rs_q * m_q, fp8
        lntmp = kside.tile([128, CT], fp32)
        bias2 = kside.tile([128, CT], fp32)

        # ---- weight DMAs first: M_T and the se matmuls only need these ----
        for co in range(CT):
            nc.sync.dma_start(out=swT8[:, co, :], in_=swT_d[co * 128 : (co + 1) * 128, :])
            nc.sync.dma_start(out=qw8[:, co, :], in_=qw8_d[co * 128 : (co + 1) * 128, :])
            nc.sync.dma_start(out=kw8[:, co, :], in_=kw8_d[co * 128 : (co + 1) * 128, :])

        # ---- q stats: shard-local bn_stats + cross-core AllReduce ----
        qsh_f = setup.tile([128, CT, NSH], fp32, name="qsh_f", bufs=1)
        mv_loc = setup.tile([128, CT, 2], fp32, name="mv_loc", bufs=1)
        part = setup.tile([128, CT, 2], fp32, name="part", bufs=1)
        red = setup.tile([128, CT, 2], fp32, name="red", bufs=1)
        for ci in range(CT):
            for s in range(NSH // 512):
                nc.sync.dma_start(
                    out=qsh_f[:, ci, s * 512 : (s + 1) * 512],
                    in_=qT_sh[ci * 128 : (ci + 1) * 128, s * 512 : (s + 1) * 512],
                )
                nc.vector.bn_stats(
                    out=stat_q[:, ci, s, :],
                    in_=qsh_f[:, ci, s * 512 : (s + 1) * 512],
                )
            nc.vector.bn_aggr(out=mv_loc[:, ci, :], in_=stat_q[:, ci])
            # part = [m_i, v_i + m_i^2]
            nc.vector.tensor_mul(
                out=part[:, ci, 0:1], in0=mv_loc[:, ci, 0:1], in1=mv_loc[:, ci, 0:1]
            )
            nc.vector.tensor_add(
                out=part[:, ci, 1:2], in0=mv_loc[:, ci, 1:2], in1=part[:, ci, 0:1]
            )
            nc.vector.tensor_copy(out=part[:, ci, 0:1], in_=mv_loc[:, ci, 0:1])
            nc.sync.dma_start(
                out=cc_in[ci * 128 : (ci + 1) * 128, :], in_=part[:, ci, :]
            )
        if sim_local:
            # CoreSim can't run the 8-core collective: fake it with a
            # DRAM->DRAM copy (stats become shard-local).
            nc.sync.dma_start(out=cc_out[:], in_=cc_in[:])
        else:
            nc.gpsimd.collective_compute(
                "AllReduce",
                ALU.add,
                replica_groups=[list(range(NCORES))],
                ins=[cc_in[:]],
                outs=[cc_out[:]],
            )

        # ---- consts + weight DMAs ----
        make_identity(nc, ident)
        nc.vector.memset(ones1, 1.0)
        nc.vector.memset(eps_t, EPS)
        nc.vector.memset(nsh_t, -ESHIFT)
        nc.sync.dma_start(out=sb_bc, in_=bcast128(sb_d[:]))
        for co in range(CT):
            nc.sync.dma_start(
                out=qb_sb[:, co : co + 1],
                in_=qb_d[co * 128 : (co + 1) * 128].rearrange("(p o) -> p o", o=1),
            )

        # ---- M_T = qw8.T-contract-kw8 (no stats dependency; runs at ~2us).
        # M_T[c'(part), c] = sum_co qw[co, c'] * kw[co, c]; evac on ACT ----
        for co in range(CT):
            mt_ps = ps_rot.tile([128, 512], fp32, name="mt_ps", tag="rot")
            nc.tensor.matmul(
                mt_ps[:, 0:C],
                qw8[:, :, co * 128 : (co + 1) * 128],
                kw8,
                start=True,
                stop=True,
                perf_mode=DR,
            )
            nc.scalar.copy(out=MT8[:, co, :], in_=mt_ps[:, 0:C])
        nc.vector.tensor_copy(out=qb8[:, :, 0], in_=qb_sb)

        # ---- st DMA (chunk-major so both ci halves of early hw chunks land
        # first -> se matmuls can start at ~2us) + stats (DVE) ----
        for ch in range(HW // SCH):
            for ci in range(CT):
                nc.sync.dma_start(
                    out=st8[:, ci, ch * SCH : (ch + 1) * SCH],
                    in_=st_d[ci * 128 : (ci + 1) * 128, ch * SCH : (ch + 1) * SCH],
                )
        for ci in range(CT):
            for ch in range(HW // SCH):
                for s in range(SCH // 512):
                    nc.vector.bn_stats(
                        out=stat_k[:, ci, ch * (SCH // 512) + s, :],
                        in_=st8[:, ci, ch * SCH + s * 512 : ch * SCH + (s + 1) * 512],
                    )
            nc.vector.bn_aggr(out=mv_k[:, ci, :], in_=stat_k[:, ci])

        # ---- style production: se matmuls (PE) -> W2 copies (ACT, busy in
        # the window where DVE is stuck on stats) -> squares (Pool) ----
        for t in range(HP):
            se_ps = ps_rot.tile([128, 2, C], fp32, name="se_ps", tag="rot")
            for half in range(2):
                h = 2 * t + half
                nc.tensor.matmul(
                    se_ps[:, half, :],
                    st8[:, :, h * 128 : (h + 1) * 128],
                    swT8,
                    start=True,
                    stop=True,
                    perf_mode=DR,
                )
            nc.scalar.copy(out=W2[:, t, :, 0:256], in_=se_ps)
            nc.gpsimd.tensor_mul(
                out=W2[:, t, :, 256:512],
                in0=W2[:, t, :, 0:256],
                in1=W2[:, t, :, 0:256],
            )

        # ---- q side (post-collective): global stats -> rs_q, rs_k ----
        for ci in range(CT):
            nc.sync.dma_start(
                out=red[:, ci, :], in_=cc_out[ci * 128 : (ci + 1) * 128, :]
            )
            inv_n = 1.0 if sim_local else 1.0 / NCORES
            nc.vector.tensor_scalar_mul(
                out=mv_q[:, ci, 0:1], in0=red[:, ci, 0:1], scalar1=inv_n
            )
            nc.vector.tensor_scalar_mul(
                out=mv_q[:, ci, 1:2], in0=red[:, ci, 1:2], scalar1=inv_n
            )
            nc.vector.tensor_mul(
                out=red[:, ci, 0:1], in0=mv_q[:, ci, 0:1], in1=mv_q[:, ci, 0:1]
            )
            nc.vector.tensor_sub(
                out=mv_q[:, ci, 1:2], in0=mv_q[:, ci, 1:2], in1=red[:, ci, 0:1]
            )
        for ci in range(CT):
            nc.scalar.activation(
                out=lntmp[:, ci : ci + 1], in_=mv_k[:, ci, 1:2], func=AF.Ln, bias=eps_t
            )
            nc.scalar.activation(
                out=rs_k[:, ci : ci + 1], in_=lntmp[:, ci : ci + 1], func=AF.Exp, scale=-0.5
            )
        for ci in range(CT):
            nc.scalar.activation(
                out=lntmp[:, ci : ci + 1], in_=mv_q[:, ci, 1:2], func=AF.Ln, bias=eps_t
            )
            nc.scalar.activation(
                out=rs_q[:, ci : ci + 1], in_=lntmp[:, ci : ci + 1], func=AF.Exp, scale=-0.5
            )
            # mqs8 = rs_q * m_q in fp8 (for the folded bias matmul)
            nc.vector.tensor_mul(
                out=red[:, ci, 1:2], in0=mv_q[:, ci, 0:1], in1=rs_q[:, ci : ci + 1]
            )
            nc.vector.tensor_copy(out=mqs8[:, ci, :], in_=red[:, ci, 1:2])

        # ---- bias2 = kw.T @ qb - M_T.T @ (rs_q*m_q)  (two F=1 chains) ----
        b_ps = ps_rot.tile([128, 512], fp32, name="b_ps", tag="rot")
        for co in range(CT):
            nc.tensor.matmul(
                b_ps[:, co : co + 1],
                kw8[:, :, co * 128 : (co + 1) * 128],
                qb8,
                start=(co == 0),
                stop=False,
                skip_group_check=True,
                perf_mode=DR,
            )
        for co in range(CT):
            nc.tensor.matmul(
                b_ps[:, CT + co : CT + co + 1],
                MT8[:, :, co * 128 : (co + 1) * 128],
                mqs8,
                start=False,
                stop=(co == CT - 1),
                skip_group_check=True,
                perf_mode=DR,
            )
        b_sb = setup.tile([128, 2 * CT], fp32, name="b_sb", bufs=1)
        nc.vector.tensor_copy(out=b_sb, in_=b_ps[:, 0 : 2 * CT])
        nc.vector.tensor_sub(out=bias2, in0=b_sb[:, 0:CT], in1=b_sb[:, CT : 2 * CT])

        # ---- q shard: fold rs_q into the fp8 cast; qn stays fp32 ----
        qsh8 = setup.tile([128, CT, NSH], f8, name="qsh8", bufs=1)
        qnT = setup.tile([128, CT, NSH], fp32, name="qnT", bufs=1)
        for ci in range(CT):
            nc.vector.tensor_scalar_mul(
                out=qsh8[:, ci, :], in0=qsh_f[:, ci, :], scalar1=rs_q[:, ci : ci + 1]
            )
            nc.vector.tensor_scalar(
                out=qnT[:, ci, :],
                in0=qsh_f[:, ci, :],
                scalar1=mv_q[:, ci, 0:1],
                scalar2=rs_q[:, ci : ci + 1],
                op0=ALU.subtract,
                op1=ALU.mult,
            )

        # ---- qk = rs_k * (M_T.T @ qsh8 + bias2)  (fp8, (C, NSH)) ----
        for co in range(CT):
            for nn in range(NSH // 512):
                qk_ps = ps_rot.tile([128, 512], fp32, name="qk_ps", tag="rot")
                nc.tensor.matmul(
                    qk_ps,
                    MT8[:, :, co * 128 : (co + 1) * 128],
                    qsh8[:, :, nn * 512 : (nn + 1) * 512],
                    start=True,
                    stop=True,
                    perf_mode=DR,
                )
                nc.vector.tensor_scalar(
                    out=qk8[:, co, nn * 512 : (nn + 1) * 512],
                    in0=qk_ps,
                    scalar1=bias2[:, co : co + 1],
                    scalar2=rs_k[:, co : co + 1],
                    op0=ALU.add,
                    op1=ALU.mult,
                )

        # ---- pre-transpose qn to natural (n, c) layout (fp32 PE) ----
        for tp in range(NSH // 256):
            qt_ps = ps_rot.tile([128, 512], fp32, name="qt_ps", tag="rot")
            for half in range(2):
                t = tp * 2 + half
                for ci in range(CT):
                    nc.tensor.transpose(
                        qt_ps[:, half * 256 + ci * 128 : half * 256 + (ci + 1) * 128],
                        qnT[:, ci, t * 128 : (t + 1) * 128],
                        ident,
                    )
            nc.vector.tensor_copy(
                out=qn_nat[:, tp * 2 : tp * 2 + 2, :], in_=qt_ps
            )

        # ================= main loop =================
        for g in range(NG):
            mv = mvps.tile([128, NJ, 512], fp32, name="mv")
            den = denps.tile([128, NJ], fp32, name="den")

            def mm1(h):
                lg = ps_rot.tile([128, 512], fp32, name="lg", tag="rot")
                nc.tensor.matmul(
                    lg,
                    st8[:, :, h * 128 : (h + 1) * 128],
                    qk8[:, :, g * 512 : (g + 1) * 512],
                    start=True,
                    stop=True,
                    perf_mode=DR,
                )
                return lg

            # software pipeline, two deep: logits mms run two h ahead of
            # the values mms so exp always has a finished bank waiting.
            pend = [mm1(0), mm1(1)]
            E_pair = None
            for h in range(HT):
                lg = pend.pop(0)
                if h % 2 == 0:
                    E_pair = epool.tile([128, 2, 512], f8, name="E_pair")
                nc.scalar.activation(
                    out=E_pair[:, h % 2, :], in_=lg, func=AF.Exp, scale=SCALE,
                    bias=nsh_t,
                )
                if h + 2 < HT:
                    pend.append(mm1(h + 2))
                if h % 2 == 1:
                    hp = h // 2
                    for j in range(NJ):
                        nc.tensor.matmul(
                            mv[:, j, :],
                            E_pair[:, :, j * 128 : (j + 1) * 128],
                            W2[:, hp, :, :],
                            start=(hp == 0),
                            stop=(hp == HP - 1),
                            perf_mode=DR,
                        )
                        # denominator: F=1 matmul with the same stationary;
                        # 4 interleaved accumulation streams share one bank
                        # (only the very first uses start=True).
                        nc.tensor.matmul(
                            den[:, j : j + 1],
                            E_pair[:, :, j * 128 : (j + 1) * 128],
                            ones1,
                            start=(hp == 0 and j == 0),
                            stop=(hp == HP - 1 and j == NJ - 1),
                            skip_group_check=True,
                            perf_mode=DR,
                        )

            # ---- drain PSUM accumulators on DVE ----
            mv_sb = ep.tile([128, NJ, 512], fp32, name="mv_sb")
            for j in range(NJ):
                nc.vector.tensor_copy(out=mv_sb[:, j, :], in_=mv[:, j, :])
            inv = ep.tile([128, NJ], fp32, name="inv")
            nc.vector.reciprocal(out=inv, in_=den)

            # ---- epilogue, elementwise ops batched across the 4 subtiles ----
            mean_a = ep.tile([128, NJ, C], fp32, name="mean_a")
            var_a = ep.tile([128, NJ, C], fp32, name="var_a")
            for j in range(NJ):
                nc.vector.tensor_scalar_mul(
                    out=mean_a[:, j, :], in0=mv_sb[:, j, 0:C], scalar1=inv[:, j : j + 1]
                )
                nc.vector.tensor_scalar_mul(
                    out=var_a[:, j, :], in0=mv_sb[:, j, C : 2 * C], scalar1=inv[:, j : j + 1]
                )
            msq = ep.tile([128, NJ, C], fp32, name="msq")
            nc.vector.tensor_mul(out=msq, in0=mean_a, in1=mean_a)
            nc.vector.tensor_sub(out=var_a, in0=var_a, in1=msq)
            nc.vector.tensor_scalar_max(out=var_a, in0=var_a, scalar1=0.0)
            # std = exp(0.5*ln(var)): stays in the exp/ln ACT table set
            std_a = ep.tile([128, NJ, C], fp32, name="std_a")
            nc.scalar.activation(out=std_a, in_=var_a, func=AF.Ln)
            nc.scalar.activation(out=std_a, in_=std_a, func=AF.Exp, scale=0.5)
            # mean of (se + sb) = raw mean + sb (var is shift-invariant)
            for j in range(NJ):
                nc.vector.tensor_add(out=mean_a[:, j, :], in0=mean_a[:, j, :], in1=sb_bc)
            cs = outp.tile([128, NJ, C], fp32, name="cs")
            nc.vector.tensor_mul(out=cs, in0=qn_nat[:, g * NJ : (g + 1) * NJ, :], in1=std_a)
            nc.vector.tensor_add(out=cs, in0=cs, in1=mean_a)
            nc.sync.dma_start(
                out=out_d[g * 512 : (g + 1) * 512, :].rearrange(
                    "(t p) c -> p t c", p=128
                ),
                in_=cs,
            )

    nc.compile()
    return nc


_cache = {}


def _get_nc():
    if "nc" not in _cache:
        _cache["nc"] = build()
    return _cache["nc"]


def _to_f8(a):
    return np.clip(np.ascontiguousarray(a, np.float32), -240.0, 240.0).astype(_f8_np)


def make_in_maps(q, k, qw, qb, kw, kb, sw, sb):
    qT = np.ascontiguousarray(q.T.astype(np.float32))
    base = {
        "st": _to_f8(k.reshape(C, HW)),
        "qw8": _to_f8(qw),
        "kw8": _to_f8(kw),
        "swT": _to_f8(sw.T),
        "qb": np.ascontiguousarray(qb.astype(np.float32)),
        "sb": np.ascontiguousarray(sb.astype(np.float32)),
    }
    return [
        {**base, "qT_sh": np.ascontiguousarray(qT[:, i * NSH : (i + 1) * NSH])}
        for i in range(NCORES)
    ]


def kernel(q, k, qw, qb, kw, kb, sw, sb):
    from concourse.bass_utils import run_bass_kernel_spmd

    q, k, qw, qb, kw, kb, sw, sb = (
        np.asarray(a) for a in (q, k, qw, qb, kw, kb, sw, sb)
    )
    nc = _get_nc()
    in_maps = make_in_maps(q, k, qw, qb, kw, kb, sw, sb)
    res = run_bass_kernel_spmd(nc, in_maps, core_ids=list(range(NCORES)))
    out = np.concatenate([res.results[i]["out"] for i in range(NCORES)], axis=0)
    return out.astype(np.float32)
